# revision 9
# baseline (speedup 1.0000x reference)
"""CrossAttention kernel for 8 Trainium2 NeuronCores (data-parallel over batch).

Math (per batch b, head h):
    q = query @ (Wq*SCALE); k = key @ Wk          (fp8 e4m3 DoubleRow matmuls,
                                                   weights pre-scaled by AQ/AK
                                                   to dodge e4m3 denormals)
    v = value @ Wv                                 (fp16)
    S^T[sk,sq] = k8_h^T q8_h                       (fp8 DoubleRow, 2x32 k-tiles)
    P = exp(S^T / (AQ*AK)) * EM^T,  EM^T = exp(bias^T) * mask^T  (host-precomp)
    outT/denom via ones-augmented v:  [v_h | 1s]^T P -> [128, sq]   (fp16)
    attn_out^T[i,sq] = outT[0:64] / denom (rows 64.. = denom)    (DVE recip)
    out[sq,o] = attn_out^T.T @ Wo  (+ bo on host)  (fp16)

Wq/Wk columns are permuted on host so each head's 64 dims land as two
32-row blocks on the same 32 partitions (DoubleRow k-tile layout for the
scores matmul).

Engine budget per core (trn2: gpsimd cannot touch PSUM):
    PE      warmup + projections + scores + attn@v + out-proj
    Scalar  exp (with 1/(AQ*AK) fold) + vaug PSUM->SBUF copies
    Vector  q8/k8 PSUM->fp8 casts, reciprocal, attn_oT divide, osb copies
    GpSimd  PT = ex*em multiply, vaug ones memset
"""
import os
import sys

import numpy as np

sys.path.insert(0, "/opt/trn_rl_repo")

import ml_dtypes

from concourse import bacc, bass, mybir, tile
from concourse.alu_op_type import AluOpType
from concourse.bass_utils import run_bass_kernel_spmd

B, S, D = 32, 512, 512
H, HD = 8, 64
NCORES = 8
BPC = B // NCORES  # batches per core
SCALE = 1.0 / np.sqrt(HD)

FP16 = mybir.dt.float16
FP8 = mybir.dt.float8e4
F32 = mybir.dt.float32
NP_FP8 = ml_dtypes.float8_e4m3

# 0 = all fp16; 1 = fp8 scores only; 2 = fp8 q/k projections + fp8 scores
FP8_MODE = 2
AQ = 64.0  # host pre-scale on Wq*SCALE (keeps e4m3 out of denormals)
AK = 16.0  # host pre-scale on Wk
UNSCALE = 1.0 / (AQ * AK) if FP8_MODE == 2 else 1.0

_last_results = None


def _build_nc():
    nc = bacc.Bacc("TRN2", target_bir_lowering=False, debug=False)

    qk_dt = FP8 if FP8_MODE == 2 else FP16
    qT = nc.dram_tensor("qT", [BPC, D, S], qk_dt, kind="ExternalInput")
    kT = nc.dram_tensor("kT", [BPC, D, S], qk_dt, kind="ExternalInput")
    vT = nc.dram_tensor("vT", [BPC, D, S], FP16, kind="ExternalInput")
    em = nc.dram_tensor("em", [BPC, S, S], FP16, kind="ExternalInput")
    wq = nc.dram_tensor("wq", [D, D], qk_dt, kind="ExternalInput")
    wk = nc.dram_tensor("wk", [D, D], qk_dt, kind="ExternalInput")
    wv = nc.dram_tensor("wv", [D, D], FP16, kind="ExternalInput")
    wo = nc.dram_tensor("wo", [D, D], FP16, kind="ExternalInput")
    out = nc.dram_tensor("out", [BPC, S, S], FP16, kind="ExternalOutput")

    Exp = mybir.ActivationFunctionType.Exp
    DR = mybir.MatmulPerfMode.DoubleRow

    with tile.TileContext(nc) as tc:
        with (
            tc.tile_pool(name="wpool", bufs=1) as wpool,
            tc.tile_pool(name="iop", bufs=2) as iop,
            tc.tile_pool(name="proj", bufs=2) as proj,
            tc.tile_pool(name="attn", bufs=3) as attn,
            tc.tile_pool(name="small", bufs=4) as small,
            tc.tile_pool(name="ps_proj", bufs=2, space="PSUM") as ps_proj,
            tc.tile_pool(name="ps_s", bufs=2, space="PSUM") as ps_s,
            tc.tile_pool(name="ps_o", bufs=2, space="PSUM") as ps_o,
        ):
            # PE warm-up: ramp the tensor engine's p-state while the first
            # input DMAs are in flight. Results are discarded.
            wu = wpool.tile([1, 512], FP16, tag="wu")
            nc.gpsimd.memset(wu[:], 0.0)
            for _ in range(12):
                pw = ps_proj.tile([128, S], F32, tag="pp")
                nc.tensor.matmul(
                    pw[0:1, 0:256], wu[:, 0:1], wu[:, 0:256], start=True, stop=True
                )

            # weights resident: [d_part, d_chunk, out] layout
            w_sb = {}
            for name, drm, dt in (
                ("wq", wq, qk_dt),
                ("wk", wk, qk_dt),
                ("wv", wv, FP16),
                ("wo", wo, FP16),
            ):
                w_sb[name] = wpool.tile([128, 4, D], dt, tag=name, name=name)

            def load_w(name, drm):
                nc.sync.dma_start(
                    w_sb[name][:], drm.ap().rearrange("(c p) i -> p c i", p=128)
                )

            for b in range(BPC):
                # ---- load inputs for this batch ----
                # For b==0 interleave with weight DMAs in first-use order so
                # the first projection matmul starts as early as possible.
                qT_sb = iop.tile([128, 4, S], qk_dt, tag="qT")
                kT_sb = iop.tile([128, 4, S], qk_dt, tag="kT")
                vT_sb = iop.tile([128, 4, S], FP16, tag="vT")
                em_sb = iop.tile([128, 4, S], FP16, tag="em")
                if b == 0:
                    load_w("wq", wq)
                nc.sync.dma_start(qT_sb[:], qT[b].rearrange("(c p) s -> p c s", p=128))
                if b == 0:
                    load_w("wk", wk)
                nc.sync.dma_start(kT_sb[:], kT[b].rearrange("(c p) s -> p c s", p=128))
                if b == 0:
                    load_w("wv", wv)
                nc.sync.dma_start(vT_sb[:], vT[b].rearrange("(c p) s -> p c s", p=128))
                nc.sync.dma_start(em_sb[:], em[b].rearrange("(c p) q -> p c q", p=128))
                if b == 0:
                    load_w("wo", wo)

                # ---- q/k projections -> fp8 head-blocked layout ----
                # With host-permuted Wq/Wk columns, psum rows of it-chunk:
                #   it=0: heads 0-3 dims 0-31   -> q8a slot 0
                #   it=1: heads 4-7 dims 0-31   -> q8b slot 0
                #   it=2: heads 0-3 dims 32-63  -> q8a slot 1
                #   it=3: heads 4-7 dims 32-63  -> q8b slot 1
                if FP8_MODE:
                    q8 = [
                        proj.tile([128, 2, S], FP8, tag=t, name=t)
                        for t in ("q8a", "q8b")
                    ]
                    k8 = [
                        proj.tile([128, 2, S], FP8, tag=t, name=t)
                        for t in ("k8a", "k8b")
                    ]
                    qk_dsts = ((q8, w_sb["wq"], qT_sb), (k8, w_sb["wk"], kT_sb))
                else:
                    qTp = proj.tile([128, 4, S], FP16, tag="qTp")
                    kTp = proj.tile([128, 4, S], FP16, tag="kTp")
                    qk_dsts = ((qTp, w_sb["wq"], qT_sb), (kTp, w_sb["wk"], kT_sb))
                for dsts, w, src in qk_dsts:
                    for it in range(4):
                        ps = ps_proj.tile([128, S], F32, tag="pp")
                        if FP8_MODE == 2:
                            for cp in range(2):
                                nc.tensor.matmul(
                                    ps[:],
                                    w[:, 2 * cp : 2 * cp + 2, it * 128 : (it + 1) * 128],
                                    src[:, 2 * cp : 2 * cp + 2, :],
                                    start=(cp == 0),
                                    stop=(cp == 1),
                                    perf_mode=DR,
                                )
                        else:
                            for c in range(4):
                                nc.tensor.matmul(
                                    ps[:],
                                    w[:, c, it * 128 : (it + 1) * 128],
                                    src[:, c, :],
                                    start=(c == 0),
                                    stop=(c == 3),
                                )
                        if FP8_MODE:
                            nc.scalar.copy(dsts[it % 2][:, it // 2, :], ps[:])
                        else:
                            nc.scalar.copy(dsts[:, it, :], ps[:])

                # v natural + 64 ones columns (PE then broadcasts the softmax
                # denominator to partitions 64..127 for free): [sk_p, sk_c, h, 128]
                vaug = proj.tile([128, 4, H, 2 * HD], FP16, tag="vaug")
                for st in range(4):
                    ps = ps_proj.tile([128, S], F32, tag="pp")
                    for c in range(4):
                        nc.tensor.matmul(
                            ps[:],
                            vT_sb[:, c, st * 128 : (st + 1) * 128],
                            w_sb["wv"][:, c, :],
                            start=(c == 0),
                            stop=(c == 3),
                        )
                    nc.scalar.copy(
                        vaug[:, st, :, 0:HD], ps[:].rearrange("p (h e) -> p h e", h=H)
                    )
                    nc.gpsimd.memset(vaug[:, st, :, HD : 2 * HD], 1.0)

                # ---- attention per head ----
                attn_oT = attn.tile([128, 4, S], FP16, tag="attn_oT")
                for h in range(H):
                    ic, po = h // 2, (h % 2) * 64
                    blk, p0 = h // 4, (h % 4) * 32
                    # P[sk, sq] = exp(k_h^T q_h / (AQ*AK)) * EM^T; exp/mult
                    # batched over two sk-chunks (1024 free) to amortize
                    # per-op overhead
                    PT = attn.tile([128, 4, S], FP16, tag="PT")
                    for sp in range(2):
                        ps = ps_s.tile([128, 2 * S], F32, tag="sc")
                        for j in range(2):
                            st = 2 * sp + j
                            if FP8_MODE:
                                nc.tensor.matmul(
                                    ps[:, j * S : (j + 1) * S],
                                    k8[blk][p0 : p0 + 32, :, st * 128 : (st + 1) * 128],
                                    q8[blk][p0 : p0 + 32, :, :],
                                    start=True,
                                    stop=True,
                                    perf_mode=DR,
                                    tile_position=(p0, 0),
                                )
                            else:
                                nc.tensor.matmul(
                                    ps[:, j * S : (j + 1) * S],
                                    kTp[po : po + 64, ic, st * 128 : (st + 1) * 128],
                                    qTp[po : po + 64, ic, :],
                                    start=True,
                                    stop=True,
                                )
                        ex = small.tile([128, 2, S], FP16, tag="ex")
                        nc.scalar.activation(
                            ex[:],
                            ps[:].rearrange("p (j s) -> p j s", j=2),
                            Exp,
                            scale=UNSCALE,
                        )
                        # Pool's single SBUF port makes tensor_tensor ~3x
                        # slower than DVE; split the P multiply so neither
                        # engine becomes the kernel bottleneck.
                        eng = nc.vector if (2 * h + sp) % 8 < 3 else nc.gpsimd
                        eng.tensor_tensor(
                            PT[:, 2 * sp : 2 * sp + 2, :],
                            ex[:],
                            em_sb[:, 2 * sp : 2 * sp + 2, :],
                            op=AluOpType.mult,
                        )
                    # [v_h | 1s]^T @ P -> [128, sq]; rows 64.. all hold the denom
                    pso = ps_o.tile([128, S], F32, tag="ov")
                    for c in range(4):
                        nc.tensor.matmul(
                            pso[:],
                            vaug[:, c, h, :],
                            PT[:, c, :],
                            start=(c == 0),
                            stop=(c == 3),
                        )
                    # reciprocal_approx_fast reads garbage from PSUM (hw);
                    # stage the denominator rows through SBUF first.
                    dcp = small.tile([64, S], F32, tag="dcp")
                    nc.vector.tensor_copy(dcp[:], pso[HD : 2 * HD, :])
                    rd = small.tile([64, S], F32, tag="rd")
                    nc.vector.reciprocal_approx_fast(rd[:], dcp[:])
                    nc.vector.tensor_tensor(
                        attn_oT[po : po + 64, ic, :],
                        pso[0:HD, :],
                        rd[:],
                        op=AluOpType.mult,
                    )

                # ---- output projection; fp16 out halves the writeback DMA ----
                for t in range(4):
                    pf = ps_o.tile([128, S], F32, tag="ov")
                    for c in range(4):
                        nc.tensor.matmul(
                            pf[:],
                            attn_oT[:, c, t * 128 : (t + 1) * 128],
                            w_sb["wo"][:, c, :],
                            start=(c == 0),
                            stop=(c == 3),
                        )
                    osb = small.tile([128, S], FP16, tag="osb")
                    nc.vector.tensor_copy(osb[:], pf[:])
                    nc.sync.dma_start(out[b, t * 128 : (t + 1) * 128, :], osb[:])

    nc.compile()
    return nc


def _perm_cols():
    """Column order so head h's dims land as 32-row blocks: it-chunks are
    (heads 0-3 lo), (heads 4-7 lo), (heads 0-3 hi), (heads 4-7 hi)."""
    perm = []
    for half in (0, 32):
        for hs in (0, 4):
            for h in range(hs, hs + 4):
                perm.extend(range(h * HD + half, h * HD + half + 32))
    return np.array(perm)


def kernel(query, key, value, mask, Wq, Wk, Wv, Wo, bo, rel_pos_emb):
    global _last_results
    query = np.asarray(query)
    key = np.asarray(key)
    value = np.asarray(value)
    mask = np.asarray(mask)

    np_qk = NP_FP8 if FP8_MODE == 2 else np.float16
    qT = np.ascontiguousarray(query.transpose(0, 2, 1)).astype(np_qk)
    kT = np.ascontiguousarray(key.transpose(0, 2, 1)).astype(np_qk)
    vT = np.ascontiguousarray(value.astype(np.float16).transpose(0, 2, 1))
    ebT = np.exp(np.asarray(rel_pos_emb)[:S, :S].T.astype(np.float32))
    em = np.ascontiguousarray(
        (ebT[None, :, :] * mask.transpose(0, 2, 1).astype(np.float32)).astype(
            np.float16
        )
    )
    wq_f = np.asarray(Wq, dtype=np.float32) * SCALE
    wk_f = np.asarray(Wk, dtype=np.float32)
    if FP8_MODE == 2:
        wq_f = wq_f * AQ
        wk_f = wk_f * AK
    if FP8_MODE:
        perm = _perm_cols()
        wq_f = np.ascontiguousarray(wq_f[:, perm])
        wk_f = np.ascontiguousarray(wk_f[:, perm])
    wq_f = wq_f.astype(np_qk)
    wk_f = wk_f.astype(np_qk)
    wv = np.asarray(Wv).astype(np.float16)
    wo = np.asarray(Wo).astype(np.float16)

    nc = _build_nc()
    in_maps = [
        {
            "qT": qT[i * BPC : (i + 1) * BPC],
            "kT": kT[i * BPC : (i + 1) * BPC],
            "vT": vT[i * BPC : (i + 1) * BPC],
            "em": em[i * BPC : (i + 1) * BPC],
            "wq": wq_f,
            "wk": wk_f,
            "wv": wv,
            "wo": wo,
        }
        for i in range(NCORES)
    ]
    trace = bool(int(os.environ.get("BASS_KERNEL_TRACE", "0")))
    res = run_bass_kernel_spmd(nc, in_maps, list(range(NCORES)), trace=trace)
    _last_results = res
    out = np.concatenate([res.results[i]["out"] for i in range(NCORES)], axis=0)
    return out.astype(np.float32) + np.asarray(bo)[None, None, :].astype(np.float32)


# revision 11
# speedup vs baseline: 1.0118x; 1.0118x over previous
"""CrossAttention kernel for 8 Trainium2 NeuronCores (data-parallel over batch).

Math (per batch b, head h):
    q = query @ (Wq*SCALE); k = key @ Wk          (fp8 e4m3 DoubleRow matmuls,
                                                   weights pre-scaled by AQ/AK
                                                   to dodge e4m3 denormals)
    v = value @ Wv                                 (fp16)
    S^T[sk,sq] = k8_h^T q8_h                       (fp8 DoubleRow, 2x32 k-tiles)
    P = exp(S^T / (AQ*AK)) * EM^T,  EM^T = exp(bias^T) * mask^T  (host-precomp)
    outT/denom via ones-augmented v:  [v_h | 1s]^T P -> [128, sq]   (fp16)
    attn_out^T[i,sq] = outT[0:64] / denom (rows 64.. = denom)    (DVE recip)
    out[sq,o] = attn_out^T.T @ Wo  (+ bo on host)  (fp16)

Wq/Wk columns are permuted on host so each head's 64 dims land as two
32-row blocks on the same 32 partitions (DoubleRow k-tile layout for the
scores matmul).

Engine budget per core (trn2: gpsimd cannot touch PSUM):
    PE      warmup + projections + scores + attn@v + out-proj
    Scalar  exp (with 1/(AQ*AK) fold) + vaug PSUM->SBUF copies
    Vector  q8/k8 PSUM->fp8 casts, reciprocal, attn_oT divide, osb copies
    GpSimd  PT = ex*em multiply, vaug ones memset
"""
import os
import sys

import numpy as np

sys.path.insert(0, "/opt/trn_rl_repo")

import ml_dtypes

from concourse import bacc, bass, mybir, tile
from concourse.alu_op_type import AluOpType
from concourse.bass_utils import run_bass_kernel_spmd

B, S, D = 32, 512, 512
H, HD = 8, 64
NCORES = 8
BPC = B // NCORES  # batches per core
SCALE = 1.0 / np.sqrt(HD)

FP16 = mybir.dt.float16
FP8 = mybir.dt.float8e4
F32 = mybir.dt.float32
NP_FP8 = ml_dtypes.float8_e4m3

# 0 = all fp16; 1 = fp8 scores only; 2 = fp8 q/k projections + fp8 scores
FP8_MODE = 2
AQ = 64.0  # host pre-scale on Wq*SCALE (keeps e4m3 out of denormals)
AK = 16.0  # host pre-scale on Wk
UNSCALE = 1.0 / (AQ * AK) if FP8_MODE == 2 else 1.0

_last_results = None


def _build_nc():
    nc = bacc.Bacc("TRN2", target_bir_lowering=False, debug=False)

    qk_dt = FP8 if FP8_MODE == 2 else FP16
    qT = nc.dram_tensor("qT", [BPC, D, S], qk_dt, kind="ExternalInput")
    kT = nc.dram_tensor("kT", [BPC, D, S], qk_dt, kind="ExternalInput")
    vT = nc.dram_tensor("vT", [BPC, D, S], FP16, kind="ExternalInput")
    em = nc.dram_tensor("em", [BPC, S, S], FP16, kind="ExternalInput")
    wq = nc.dram_tensor("wq", [D, D], qk_dt, kind="ExternalInput")
    wk = nc.dram_tensor("wk", [D, D], qk_dt, kind="ExternalInput")
    wv = nc.dram_tensor("wv", [D, D], FP16, kind="ExternalInput")
    wo = nc.dram_tensor("wo", [D, D], FP16, kind="ExternalInput")
    out = nc.dram_tensor("out", [BPC, S, S], FP16, kind="ExternalOutput")

    Exp = mybir.ActivationFunctionType.Exp
    DR = mybir.MatmulPerfMode.DoubleRow

    with tile.TileContext(nc) as tc:
        with (
            tc.tile_pool(name="wpool", bufs=1) as wpool,
            tc.tile_pool(name="iop", bufs=2) as iop,
            tc.tile_pool(name="proj", bufs=2) as proj,
            tc.tile_pool(name="attn", bufs=3) as attn,
            tc.tile_pool(name="small", bufs=4) as small,
            tc.tile_pool(name="ps_proj", bufs=2, space="PSUM") as ps_proj,
            tc.tile_pool(name="ps_s", bufs=2, space="PSUM") as ps_s,
            tc.tile_pool(name="ps_o", bufs=2, space="PSUM") as ps_o,
        ):
            # PE warm-up: ramp the tensor engine's p-state while the first
            # input DMAs are in flight. Results are discarded.
            wu = wpool.tile([1, 512], FP16, tag="wu")
            nc.gpsimd.memset(wu[:], 0.0)
            for _ in range(12):
                pw = ps_proj.tile([128, S], F32, tag="pp")
                nc.tensor.matmul(
                    pw[0:1, 0:256], wu[:, 0:1], wu[:, 0:256], start=True, stop=True
                )

            # weights resident: [d_part, d_chunk, out] layout
            w_sb = {}
            for name, drm, dt in (
                ("wq", wq, qk_dt),
                ("wk", wk, qk_dt),
                ("wv", wv, FP16),
                ("wo", wo, FP16),
            ):
                w_sb[name] = wpool.tile([128, 4, D], dt, tag=name, name=name)

            def load_w(name, drm):
                nc.sync.dma_start(
                    w_sb[name][:], drm.ap().rearrange("(c p) i -> p c i", p=128)
                )

            for b in range(BPC):
                # ---- load inputs for this batch ----
                # For b==0 interleave with weight DMAs in first-use order so
                # the first projection matmul starts as early as possible.
                qT_sb = iop.tile([128, 4, S], qk_dt, tag="qT")
                kT_sb = iop.tile([128, 4, S], qk_dt, tag="kT")
                vT_sb = iop.tile([128, 4, S], FP16, tag="vT")
                em_sb = iop.tile([128, 4, S], FP16, tag="em")
                if b == 0:
                    load_w("wq", wq)
                nc.sync.dma_start(qT_sb[:], qT[b].rearrange("(c p) s -> p c s", p=128))
                if b == 0:
                    load_w("wk", wk)
                nc.sync.dma_start(kT_sb[:], kT[b].rearrange("(c p) s -> p c s", p=128))
                if b == 0:
                    load_w("wv", wv)
                nc.sync.dma_start(vT_sb[:], vT[b].rearrange("(c p) s -> p c s", p=128))
                nc.sync.dma_start(em_sb[:], em[b].rearrange("(c p) q -> p c q", p=128))
                if b == 0:
                    load_w("wo", wo)

                # ---- q/k projections -> fp8 head-blocked layout ----
                # With host-permuted Wq/Wk columns, psum rows of it-chunk:
                #   it=0: heads 0-3 dims 0-31   -> q8a slot 0
                #   it=1: heads 4-7 dims 0-31   -> q8b slot 0
                #   it=2: heads 0-3 dims 32-63  -> q8a slot 1
                #   it=3: heads 4-7 dims 32-63  -> q8b slot 1
                if FP8_MODE:
                    q8 = [
                        proj.tile([128, 2, S], FP8, tag=t, name=t)
                        for t in ("q8a", "q8b")
                    ]
                    k8 = [
                        proj.tile([128, 2, S], FP8, tag=t, name=t)
                        for t in ("k8a", "k8b")
                    ]
                    qk_dsts = ((q8, w_sb["wq"], qT_sb), (k8, w_sb["wk"], kT_sb))
                else:
                    qTp = proj.tile([128, 4, S], FP16, tag="qTp")
                    kTp = proj.tile([128, 4, S], FP16, tag="kTp")
                    qk_dsts = ((qTp, w_sb["wq"], qT_sb), (kTp, w_sb["wk"], kT_sb))
                for dsts, w, src in qk_dsts:
                    for it in range(4):
                        ps = ps_proj.tile([128, S], F32, tag="pp")
                        if FP8_MODE == 2:
                            for cp in range(2):
                                nc.tensor.matmul(
                                    ps[:],
                                    w[:, 2 * cp : 2 * cp + 2, it * 128 : (it + 1) * 128],
                                    src[:, 2 * cp : 2 * cp + 2, :],
                                    start=(cp == 0),
                                    stop=(cp == 1),
                                    perf_mode=DR,
                                )
                        else:
                            for c in range(4):
                                nc.tensor.matmul(
                                    ps[:],
                                    w[:, c, it * 128 : (it + 1) * 128],
                                    src[:, c, :],
                                    start=(c == 0),
                                    stop=(c == 3),
                                )
                        if FP8_MODE:
                            nc.scalar.copy(dsts[it % 2][:, it // 2, :], ps[:])
                        else:
                            nc.scalar.copy(dsts[:, it, :], ps[:])

                # v natural + 64 ones columns (PE then broadcasts the softmax
                # denominator to partitions 64..127 for free): [sk_p, sk_c, h, 128]
                vaug = proj.tile([128, 4, H, 2 * HD], FP16, tag="vaug")
                for st in range(4):
                    ps = ps_proj.tile([128, S], F32, tag="pp")
                    for c in range(4):
                        nc.tensor.matmul(
                            ps[:],
                            vT_sb[:, c, st * 128 : (st + 1) * 128],
                            w_sb["wv"][:, c, :],
                            start=(c == 0),
                            stop=(c == 3),
                        )
                    nc.scalar.copy(
                        vaug[:, st, :, 0:HD], ps[:].rearrange("p (h e) -> p h e", h=H)
                    )
                    nc.gpsimd.memset(vaug[:, st, :, HD : 2 * HD], 1.0)

                # ---- attention per head ----
                attn_oT = attn.tile([128, 4, S], FP16, tag="attn_oT")
                for h in range(H):
                    ic, po = h // 2, (h % 2) * 64
                    blk, p0 = h // 4, (h % 4) * 32
                    # P[sk, sq] = exp(k_h^T q_h / (AQ*AK)) * EM^T; exp/mult
                    # batched over two sk-chunks (1024 free) to amortize
                    # per-op overhead
                    PT = attn.tile([128, 4, S], FP16, tag="PT")
                    for sp in range(2):
                        ps = ps_s.tile([128, 2 * S], F32, tag="sc")
                        for j in range(2):
                            st = 2 * sp + j
                            if FP8_MODE:
                                nc.tensor.matmul(
                                    ps[:, j * S : (j + 1) * S],
                                    k8[blk][p0 : p0 + 32, :, st * 128 : (st + 1) * 128],
                                    q8[blk][p0 : p0 + 32, :, :],
                                    start=True,
                                    stop=True,
                                    perf_mode=DR,
                                    tile_position=(p0, 0),
                                )
                            else:
                                nc.tensor.matmul(
                                    ps[:, j * S : (j + 1) * S],
                                    kTp[po : po + 64, ic, st * 128 : (st + 1) * 128],
                                    qTp[po : po + 64, ic, :],
                                    start=True,
                                    stop=True,
                                )
                        ex = small.tile([128, 2, S], FP16, tag="ex")
                        nc.scalar.activation(
                            ex[:],
                            ps[:].rearrange("p (j s) -> p j s", j=2),
                            Exp,
                            scale=UNSCALE,
                        )
                        # Pool's single SBUF port + ~1us/instr semaphore cost
                        # make it a poor tensor_tensor engine; offload only a
                        # quarter of the P multiplies to it.
                        eng = nc.gpsimd if (2 * h + sp) % 4 == 3 else nc.vector
                        eng.tensor_tensor(
                            PT[:, 2 * sp : 2 * sp + 2, :],
                            ex[:],
                            em_sb[:, 2 * sp : 2 * sp + 2, :],
                            op=AluOpType.mult,
                        )
                    # [v_h | 1s]^T @ P -> [128, sq]; rows 64.. all hold the denom
                    pso = ps_o.tile([128, S], F32, tag="ov")
                    for c in range(4):
                        nc.tensor.matmul(
                            pso[:],
                            vaug[:, c, h, :],
                            PT[:, c, :],
                            start=(c == 0),
                            stop=(c == 3),
                        )
                    # reciprocal_approx_fast reads garbage from PSUM (hw);
                    # stage the denominator rows through SBUF first.
                    dcp = small.tile([64, S], F32, tag="dcp")
                    nc.vector.tensor_copy(dcp[:], pso[HD : 2 * HD, :])
                    rd = small.tile([64, S], F32, tag="rd")
                    nc.vector.reciprocal_approx_fast(rd[:], dcp[:])
                    nc.vector.tensor_tensor(
                        attn_oT[po : po + 64, ic, :],
                        pso[0:HD, :],
                        rd[:],
                        op=AluOpType.mult,
                    )

                # ---- output projection; fp16 out halves the writeback DMA ----
                for t in range(4):
                    pf = ps_o.tile([128, S], F32, tag="ov")
                    for c in range(4):
                        nc.tensor.matmul(
                            pf[:],
                            attn_oT[:, c, t * 128 : (t + 1) * 128],
                            w_sb["wo"][:, c, :],
                            start=(c == 0),
                            stop=(c == 3),
                        )
                    osb = small.tile([128, S], FP16, tag="osb")
                    nc.scalar.copy(osb[:], pf[:])
                    nc.sync.dma_start(out[b, t * 128 : (t + 1) * 128, :], osb[:])

    nc.compile()
    return nc


def _perm_cols():
    """Column order so head h's dims land as 32-row blocks: it-chunks are
    (heads 0-3 lo), (heads 4-7 lo), (heads 0-3 hi), (heads 4-7 hi)."""
    perm = []
    for half in (0, 32):
        for hs in (0, 4):
            for h in range(hs, hs + 4):
                perm.extend(range(h * HD + half, h * HD + half + 32))
    return np.array(perm)


def kernel(query, key, value, mask, Wq, Wk, Wv, Wo, bo, rel_pos_emb):
    global _last_results
    query = np.asarray(query)
    key = np.asarray(key)
    value = np.asarray(value)
    mask = np.asarray(mask)

    np_qk = NP_FP8 if FP8_MODE == 2 else np.float16
    qT = np.ascontiguousarray(query.transpose(0, 2, 1)).astype(np_qk)
    kT = np.ascontiguousarray(key.transpose(0, 2, 1)).astype(np_qk)
    vT = np.ascontiguousarray(value.astype(np.float16).transpose(0, 2, 1))
    ebT = np.exp(np.asarray(rel_pos_emb)[:S, :S].T.astype(np.float32))
    em = np.ascontiguousarray(
        (ebT[None, :, :] * mask.transpose(0, 2, 1).astype(np.float32)).astype(
            np.float16
        )
    )
    wq_f = np.asarray(Wq, dtype=np.float32) * SCALE
    wk_f = np.asarray(Wk, dtype=np.float32)
    if FP8_MODE == 2:
        wq_f = wq_f * AQ
        wk_f = wk_f * AK
    if FP8_MODE:
        perm = _perm_cols()
        wq_f = np.ascontiguousarray(wq_f[:, perm])
        wk_f = np.ascontiguousarray(wk_f[:, perm])
    wq_f = wq_f.astype(np_qk)
    wk_f = wk_f.astype(np_qk)
    wv = np.asarray(Wv).astype(np.float16)
    wo = np.asarray(Wo).astype(np.float16)

    nc = _build_nc()
    in_maps = [
        {
            "qT": qT[i * BPC : (i + 1) * BPC],
            "kT": kT[i * BPC : (i + 1) * BPC],
            "vT": vT[i * BPC : (i + 1) * BPC],
            "em": em[i * BPC : (i + 1) * BPC],
            "wq": wq_f,
            "wk": wk_f,
            "wv": wv,
            "wo": wo,
        }
        for i in range(NCORES)
    ]
    trace = bool(int(os.environ.get("BASS_KERNEL_TRACE", "0")))
    res = run_bass_kernel_spmd(nc, in_maps, list(range(NCORES)), trace=trace)
    _last_results = res
    out = np.concatenate([res.results[i]["out"] for i in range(NCORES)], axis=0)
    return out.astype(np.float32) + np.asarray(bo)[None, None, :].astype(np.float32)


# revision 12
# speedup vs baseline: 1.2068x; 1.1927x over previous
"""CrossAttention kernel for 8 Trainium2 NeuronCores (data-parallel over batch).

Math (per batch b, head h):
    q = query @ (Wq*SCALE); k = key @ Wk; v = value @ Wv        (fp16 matmuls)
    S^T[sk,sq] = k_h^T q_h                                       (per head)
    P = exp(S^T) * EM^T,  EM^T = exp(bias^T) * mask^T            (host-precomputed)
    outT/denom via ones-augmented v:  [v_h | 1s]^T P -> [128, sq]
    attn_out^T[i,sq] = outT[0:64] / denom (rows 64.. = denom)    (DVE recip)
    out[sq,o] = attn_out^T.T @ Wo  (+ bo on host)
"""
import os
import sys

import numpy as np

sys.path.insert(0, "/opt/trn_rl_repo")

from concourse import bacc, bass, mybir, tile
from concourse.alu_op_type import AluOpType
from concourse.bass_utils import run_bass_kernel_spmd

B, S, D = 32, 512, 512
H, HD = 8, 64
NCORES = 8
BPC = B // NCORES  # batches per core
SCALE = 1.0 / np.sqrt(HD)

FP16 = mybir.dt.float16
F32 = mybir.dt.float32

_last_results = None


def _build_nc():
    nc = bacc.Bacc("TRN2", target_bir_lowering=False, debug=False)

    qT = nc.dram_tensor("qT", [BPC, D, S], FP16, kind="ExternalInput")
    kT = nc.dram_tensor("kT", [BPC, D, S], FP16, kind="ExternalInput")
    vT = nc.dram_tensor("vT", [BPC, D, S], FP16, kind="ExternalInput")
    em = nc.dram_tensor("em", [BPC, S, S], FP16, kind="ExternalInput")
    wq = nc.dram_tensor("wq", [D, D], FP16, kind="ExternalInput")
    wk = nc.dram_tensor("wk", [D, D], FP16, kind="ExternalInput")
    wv = nc.dram_tensor("wv", [D, D], FP16, kind="ExternalInput")
    wo = nc.dram_tensor("wo", [D, D], FP16, kind="ExternalInput")
    out = nc.dram_tensor("out", [BPC, S, S], F32, kind="ExternalOutput")

    Exp = mybir.ActivationFunctionType.Exp

    with tile.TileContext(nc) as tc:
        with (
            tc.tile_pool(name="wpool", bufs=1) as wpool,
            tc.tile_pool(name="iop", bufs=2) as iop,
            tc.tile_pool(name="proj", bufs=2) as proj,
            tc.tile_pool(name="attn", bufs=3) as attn,
            tc.tile_pool(name="small", bufs=4) as small,
            tc.tile_pool(name="ps_proj", bufs=2, space="PSUM") as ps_proj,
            tc.tile_pool(name="ps_s", bufs=2, space="PSUM") as ps_s,
            tc.tile_pool(name="ps_o", bufs=2, space="PSUM") as ps_o,
        ):
            # weights resident: [d_part, d_chunk, out] layout
            w_sb = {}
            for name, drm in (("wq", wq), ("wk", wk), ("wv", wv), ("wo", wo)):
                t = wpool.tile([128, 4, D], FP16, tag=name)
                nc.sync.dma_start(t[:], drm.ap().rearrange("(c p) i -> p c i", p=128))
                w_sb[name] = t

            for b in range(BPC):
                # ---- load inputs for this batch ----
                qT_sb = iop.tile([128, 4, S], FP16, tag="qT")
                kT_sb = iop.tile([128, 4, S], FP16, tag="kT")
                vT_sb = iop.tile([128, 4, S], FP16, tag="vT")
                em_sb = iop.tile([128, 4, S], FP16, tag="em")
                nc.sync.dma_start(qT_sb[:], qT[b].rearrange("(c p) s -> p c s", p=128))
                nc.sync.dma_start(kT_sb[:], kT[b].rearrange("(c p) s -> p c s", p=128))
                nc.sync.dma_start(vT_sb[:], vT[b].rearrange("(c p) s -> p c s", p=128))
                nc.sync.dma_start(em_sb[:], em[b].rearrange("(c p) q -> p c q", p=128))

                # ---- projections ----
                # q^T_proj, k^T_proj: [i_part, i_chunk, sq]
                qTp = proj.tile([128, 4, S], FP16, tag="qTp")
                kTp = proj.tile([128, 4, S], FP16, tag="kTp")
                for dst, w, src in ((qTp, w_sb["wq"], qT_sb), (kTp, w_sb["wk"], kT_sb)):
                    for it in range(4):
                        ps = ps_proj.tile([128, S], F32, tag="pp")
                        for c in range(4):
                            nc.tensor.matmul(
                                ps[:],
                                w[:, c, it * 128 : (it + 1) * 128],
                                src[:, c, :],
                                start=(c == 0),
                                stop=(c == 3),
                            )
                        nc.scalar.copy(dst[:, it, :], ps[:])

                # v natural + 64 ones columns (PE then broadcasts the softmax
                # denominator to partitions 64..127 for free): [sk_p, sk_c, h, 128]
                vaug = proj.tile([128, 4, H, 2 * HD], FP16, tag="vaug")
                for st in range(4):
                    ps = ps_proj.tile([128, S], F32, tag="pp")
                    for c in range(4):
                        nc.tensor.matmul(
                            ps[:],
                            vT_sb[:, c, st * 128 : (st + 1) * 128],
                            w_sb["wv"][:, c, :],
                            start=(c == 0),
                            stop=(c == 3),
                        )
                    nc.vector.tensor_copy(
                        vaug[:, st, :, 0:HD], ps[:].rearrange("p (h e) -> p h e", h=H)
                    )
                    nc.vector.memset(vaug[:, st, :, HD : 2 * HD], 1.0)

                # ---- attention per head ----
                attn_oT = attn.tile([128, 4, S], FP16, tag="attn_oT")
                for h in range(H):
                    ic, po = h // 2, (h % 2) * 64
                    # P[sk, sq] = exp(k_h^T q_h) * EM^T; exp/mult batched over
                    # two sk-chunks (1024 free) to amortize per-op overhead
                    PT = attn.tile([128, 4, S], FP16, tag="PT")
                    for sp in range(2):
                        ps = ps_s.tile([128, 2 * S], F32, tag="sc")
                        for j in range(2):
                            st = 2 * sp + j
                            nc.tensor.matmul(
                                ps[:, j * S : (j + 1) * S],
                                kTp[po : po + 64, ic, st * 128 : (st + 1) * 128],
                                qTp[po : po + 64, ic, :],
                                start=True,
                                stop=True,
                            )
                        ex = small.tile([128, 2, S], FP16, tag="ex")
                        nc.scalar.activation(
                            ex[:], ps[:].rearrange("p (j s) -> p j s", j=2), Exp
                        )
                        nc.vector.tensor_tensor(
                            PT[:, 2 * sp : 2 * sp + 2, :],
                            ex[:],
                            em_sb[:, 2 * sp : 2 * sp + 2, :],
                            op=AluOpType.mult,
                        )
                    # [v_h | 1s]^T @ P -> [128, sq]; rows 64.. all hold the denom
                    pso = ps_o.tile([128, S], F32, tag="ov")
                    for c in range(4):
                        nc.tensor.matmul(
                            pso[:],
                            vaug[:, c, h, :],
                            PT[:, c, :],
                            start=(c == 0),
                            stop=(c == 3),
                        )
                    dcp = small.tile([64, S], F32, tag="dcp")
                    nc.vector.tensor_copy(dcp[:], pso[HD : 2 * HD, :])
                    rd = small.tile([64, S], F32, tag="rd")
                    nc.vector.reciprocal_approx_fast(rd[:], dcp[:])
                    nc.vector.tensor_tensor(
                        attn_oT[po : po + 64, ic, :],
                        pso[0:HD, :],
                        rd[:],
                        op=AluOpType.mult,
                    )

                # ---- output projection; DMA straight from PSUM ----
                for t in range(4):
                    pf = ps_o.tile([128, S], F32, tag="ov")
                    for c in range(4):
                        nc.tensor.matmul(
                            pf[:],
                            attn_oT[:, c, t * 128 : (t + 1) * 128],
                            w_sb["wo"][:, c, :],
                            start=(c == 0),
                            stop=(c == 3),
                        )
                    osb = small.tile([128, S], F32, tag="osb")
                    nc.scalar.copy(osb[:], pf[:])
                    nc.sync.dma_start(out[b, t * 128 : (t + 1) * 128, :], osb[:])

    nc.compile()
    return nc


def kernel(query, key, value, mask, Wq, Wk, Wv, Wo, bo, rel_pos_emb):
    global _last_results
    query = np.asarray(query)
    key = np.asarray(key)
    value = np.asarray(value)
    mask = np.asarray(mask)

    qT = np.ascontiguousarray(query.astype(np.float16).transpose(0, 2, 1))
    kT = np.ascontiguousarray(key.astype(np.float16).transpose(0, 2, 1))
    vT = np.ascontiguousarray(value.astype(np.float16).transpose(0, 2, 1))
    ebT = np.exp(np.asarray(rel_pos_emb)[:S, :S].T.astype(np.float32))
    em = np.ascontiguousarray(
        (ebT[None, :, :] * mask.transpose(0, 2, 1).astype(np.float32)).astype(
            np.float16
        )
    )
    wq = (np.asarray(Wq) * SCALE).astype(np.float16)
    wk = np.asarray(Wk).astype(np.float16)
    wv = np.asarray(Wv).astype(np.float16)
    wo = np.asarray(Wo).astype(np.float16)

    nc = _build_nc()
    in_maps = [
        {
            "qT": qT[i * BPC : (i + 1) * BPC],
            "kT": kT[i * BPC : (i + 1) * BPC],
            "vT": vT[i * BPC : (i + 1) * BPC],
            "em": em[i * BPC : (i + 1) * BPC],
            "wq": wq,
            "wk": wk,
            "wv": wv,
            "wo": wo,
        }
        for i in range(NCORES)
    ]
    trace = bool(int(os.environ.get("BASS_KERNEL_TRACE", "0")))
    res = run_bass_kernel_spmd(nc, in_maps, list(range(NCORES)), trace=trace)
    _last_results = res
    out = np.concatenate([res.results[i]["out"] for i in range(NCORES)], axis=0)
    return out + np.asarray(bo)[None, None, :].astype(np.float32)

